# revision 1
# baseline (speedup 1.0000x reference)
"""Trainium2 Bass kernel for nn_Attention_81037442941065.

Dual-attention module (spatial [b,h,n,n] + channel [b,h,d,d]) with
B=2, N=2048, DIM=1024, 16 heads of d=64.

Sharding: 8 cores = (2 batches) x (4 head-groups of 4 heads).
Each core computes its batch/head-group slice end-to-end and produces a
partial (over head groups) output projection; the host sums the 4 group
partials per batch (the "all-reduce after to_out") and adds b_out.

Dtypes: all matmul operands are bf16 (1 cycle/row on the PE; fp32 and
even float32r stream at ~2-4 cycles/row on real TRN2 hardware) with all
accumulation in fp32 PSUM; softmax statistics (sums/reciprocals) in fp32.
Measured end-to-end relative error ~4e-3 vs the fp32 reference.

Per-core layouts (everything "T" is [channels, tokens]):
  z1T, yhT   : [256, 2048]  (transposed projections, head h at rows 64*(h%2)
                             of tile h//2)
  xh_aug     : 16 tiles [128, 260] (natural layout per 128-token chunk;
               per head 65 cols = 64 channels + a ones column so the AV
               matmul also produces the softmax denominators)
  spatial    : S^T = yh @ z1^T computed [keys, queries]; exp on ScalarE
               (scale 1/8 fused, no max subtraction - logits are small);
               AV matmul lhsT=[xh|1] accumulates over key chunks -> rows
               0..63 = unnormalized out1^T, row 64 = sum of exp.
  channel    : [64,64] per head, one PSUM bank each; softmax via
               Exp+accum_out and per-partition reciprocal multiply.
"""

import sys

for _p in ("/opt/trn_rl_repo", "/opt/pypackages"):
    if _p not in sys.path:
        sys.path.insert(0, _p)

import ml_dtypes
import numpy as np
from contextlib import ExitStack

import concourse.bacc as bacc
import concourse.mybir as mybir
import concourse.tile as tile
from concourse.tile import add_dep_helper
from concourse.bass_utils import run_bass_kernel_spmd

F32 = mybir.dt.float32
F32R = mybir.dt.float32r
BF16 = mybir.dt.bfloat16
ATT = mybir.dt.bfloat16   # attention-internal matmul dtype
EXP = mybir.ActivationFunctionType.Exp

B, N, DIM = 2, 2048, 1024
HEADS, DH = 16, 64
G = 4              # head groups == cores per batch
HG = HEADS // G    # heads per group (4)
CIN = HG * DH      # inner channels per core (256)
NCORES = 8
KC = DIM // 128    # contraction chunks for projections (8)
NCH = N // 128     # 128-token chunks (16)
SCALE = DH ** -0.5            # 1/8
CM_SCALE = SCALE / (N / DH)   # 1/256


def _build_program():
    nc = bacc.Bacc(
        "TRN2", target_bir_lowering=False, debug=False, num_devices=NCORES
    )

    # ---- DRAM I/O ----
    xT_d = nc.dram_tensor("xT", [DIM, N], BF16, kind="ExternalInput").ap()
    yT_d = nc.dram_tensor("yT", [DIM, N], BF16, kind="ExternalInput").ap()
    zT_d = nc.dram_tensor("zT", [DIM, N], BF16, kind="ExternalInput").ap()
    wsa1_d = nc.dram_tensor("w_sa1", [DIM, CIN], BF16, kind="ExternalInput").ap()
    wsa2_d = nc.dram_tensor("w_sa2", [DIM, CIN], BF16, kind="ExternalInput").ap()
    wse1_d = nc.dram_tensor("w_se1", [DIM, CIN], BF16, kind="ExternalInput").ap()
    wse2_d = nc.dram_tensor("w_se2", [DIM, CIN], BF16, kind="ExternalInput").ap()
    wout_d = nc.dram_tensor("w_out", [CIN, DIM], ATT, kind="ExternalInput").ap()
    outT_d = nc.dram_tensor("outT", [DIM, N], F32, kind="ExternalOutput").ap()

    with tile.TileContext(nc) as tc, ExitStack() as ctx:
        ppool = ctx.enter_context(tc.tile_pool(name="persist", bufs=1))

        # Persistent projection outputs (live across both scopes).
        z1T = [ppool.tile([128, N], ATT, tag=f"z1T{m}", name=f"z1T{m}")
               for m in range(2)]
        yhT = [ppool.tile([128, N], ATT, tag=f"yhT{m}", name=f"yhT{m}")
               for m in range(2)]
        xh_aug = [ppool.tile([128, HG * (DH + 1)], ATT, tag=f"xa{i}",
                             name=f"xa{i}") for i in range(NCH)]
        secm_sb = [ppool.tile([128, DH], ATT, tag=f"cm{p}", name=f"cm{p}")
                   for p in range(2)]
        rs = [ppool.tile([64, 1], F32, tag=f"rs{h}", name=f"rs{h}")
              for h in range(HG)]
        rcm = [ppool.tile([64, 1], F32, tag=f"rcm{h}", name=f"rcm{h}")
               for h in range(HG)]

        ptpool = ctx.enter_context(tc.tile_pool(name="pt", bufs=4))
        tpool = ctx.enter_context(tc.tile_pool(name="tails", bufs=3))
        opool = ctx.enter_context(tc.tile_pool(name="oout", bufs=4))
        spool = ctx.enter_context(tc.tile_pool(name="spat", bufs=1))
        # w_out as four 64-row slices (base partition 0) matching cat4
        wq = [spool.tile([64, DIM], ATT, tag=f"wq{q}", name=f"wq{q}")
              for q in range(HG)]
        for q in range(HG):
            nc.sync.dma_start(wq[q][:], wout_d[q * 64:(q + 1) * 64, :])
        # cat^T staging: one [64, N] tile per head (this core's inner
        # channels [64h, 64h+64)); the final projection contracts them
        # with matching 64-row slices of w_out
        cat4 = [spool.tile([64, N], ATT, tag=f"cat{h}", name=f"cat{h}")
                for h in range(HG)]

        # ============ Scope 1: all projections + channel-attn logits ======
        with tc.tile_pool(name="proj_in", bufs=1) as ipool, \
             tc.tile_pool(name="psp", bufs=4, space="PSUM") as psp, \
             tc.tile_pool(name="pscm", bufs=1, space="PSUM") as pscm:
            # weights first (small), then x, z, y in consumption order
            wse1_t = [ipool.tile([128, CIN], BF16, tag=f"wse1_{k}",
                                 name=f"wse1_{k}") for k in range(KC)]
            wsa1_t = [ipool.tile([128, CIN], BF16, tag=f"wsa1_{k}",
                                 name=f"wsa1_{k}") for k in range(KC)]
            wse2_t = [ipool.tile([128, CIN], BF16, tag=f"wse2_{k}",
                                 name=f"wse2_{k}") for k in range(KC)]
            wsa2_t = [ipool.tile([128, CIN], BF16, tag=f"wsa2_{k}",
                                 name=f"wsa2_{k}") for k in range(KC)]
            xTt = [ipool.tile([128, N], BF16, tag=f"x{k}", name=f"x{k}")
                   for k in range(KC)]
            zTt = [ipool.tile([128, N], BF16, tag=f"z{k}", name=f"z{k}")
                   for k in range(KC)]
            yTt = [ipool.tile([128, N], BF16, tag=f"y{k}", name=f"y{k}")
                   for k in range(KC)]
            for k in range(KC):
                nc.sync.dma_start(wse1_t[k][:], wse1_d[k * 128:(k + 1) * 128, :])
            for k in range(KC):
                nc.sync.dma_start(xTt[k][:], xT_d[k * 128:(k + 1) * 128, :])
            for k in range(KC):
                nc.sync.dma_start(wsa1_t[k][:], wsa1_d[k * 128:(k + 1) * 128, :])
                nc.sync.dma_start(wse2_t[k][:], wse2_d[k * 128:(k + 1) * 128, :])
            for k in range(KC):
                nc.sync.dma_start(zTt[k][:], zT_d[k * 128:(k + 1) * 128, :])
            for k in range(KC):
                nc.sync.dma_start(wsa2_t[k][:], wsa2_d[k * 128:(k + 1) * 128, :])
            for k in range(KC):
                nc.sync.dma_start(yTt[k][:], yT_d[k * 128:(k + 1) * 128, :])

            cmps = [pscm.tile([64, DH], F32, tag=f"cmp{h}", name=f"cmp{h}")
                    for h in range(HG)]

            # --- xh (natural, augmented with ones) ---
            for i in range(NCH):
                ps = psp.tile([128, 512], F32, tag="pj", name=f"psx{i}")
                for k in range(KC):
                    nc.tensor.matmul(
                        ps[:, 0:CIN],
                        lhsT=xTt[k][:, i * 128:(i + 1) * 128],
                        rhs=wse1_t[k][:],
                        start=(k == 0), stop=(k == KC - 1),
                    )
                src = ps[:, 0:CIN].rearrange("p (h c) -> p h c", c=DH)
                dst = xh_aug[i][:].rearrange("p (h c) -> p h c", c=DH + 1)
                nc.vector.tensor_copy(dst[:, :, 0:DH], src)
                nc.scalar.activation(dst[:, :, DH:DH + 1], src[:, :, 0:1],
                                     mybir.ActivationFunctionType.Copy,
                                     bias=1.0, scale=0.0)

            # --- z1T (transposed projection) ---
            for m in range(2):
                for nb in range(4):
                    ps = psp.tile([128, 512], F32, tag="pj", name=f"psz{m}{nb}")
                    for k in range(KC):
                        nc.tensor.matmul(
                            ps[:],
                            lhsT=wsa1_t[k][:, m * 128:(m + 1) * 128],
                            rhs=zTt[k][:, nb * 512:(nb + 1) * 512],
                            start=(k == 0), stop=(k == KC - 1),
                        )
                    nc.scalar.copy(z1T[m][:, nb * 512:(nb + 1) * 512], ps[:])

            # --- z2 (natural, streamed) + channel-attn logits ---
            for i in range(NCH):
                ps2 = psp.tile([128, 512], F32, tag="pj", name=f"psz2_{i}")
                for k in range(KC):
                    nc.tensor.matmul(
                        ps2[:, 0:CIN],
                        lhsT=zTt[k][:, i * 128:(i + 1) * 128],
                        rhs=wse2_t[k][:],
                        start=(k == 0), stop=(k == KC - 1),
                    )
                z2n = ipool.tile([128, CIN], ATT, tag="z2n", bufs=3,
                                 name=f"z2n{i}")
                nc.scalar.copy(z2n[:], ps2[:, 0:CIN])
                for h in range(HG):
                    nc.tensor.matmul(
                        cmps[h][:],
                        lhsT=xh_aug[i][:, 65 * h:65 * h + DH],
                        rhs=z2n[:, DH * h:DH * (h + 1)],
                        start=(i == 0), stop=(i == NCH - 1),
                    )

            # --- yhT (transposed projection) ---
            for m in range(2):
                for nb in range(4):
                    ps = psp.tile([128, 512], F32, tag="pj", name=f"psy{m}{nb}")
                    for k in range(KC):
                        nc.tensor.matmul(
                            ps[:],
                            lhsT=wsa2_t[k][:, m * 128:(m + 1) * 128],
                            rhs=yTt[k][:, nb * 512:(nb + 1) * 512],
                            start=(k == 0), stop=(k == KC - 1),
                        )
                    nc.scalar.copy(yhT[m][:, nb * 512:(nb + 1) * 512], ps[:])

            # --- channel-attn softmax, DMA'd into pair-packed secm_sb ---
            for h in range(HG):
                p_, off = h // 2, 64 * (h % 2)
                st = ipool.tile([64, DH], ATT, tag="cmstage", bufs=4,
                                name=f"cmstage{h}")
                nc.scalar.activation(st[:], cmps[h][:], EXP,
                                     scale=CM_SCALE,
                                     accum_out=rs[h][0:64, 0:1])
                nc.vector.reciprocal(rcm[h][0:64, 0:1], rs[h][0:64, 0:1])
                nc.vector.tensor_scalar_mul(st[:], st[:], rcm[h][0:64, 0:1])
                nc.sync.dma_start(secm_sb[p_][off:off + 64, :], st[:])

        # ============ Scope 2: out2, spatial attention, final projection ==
        # PSUM: S tag 2x[128,1024] (4 banks) + av 2x[128,512] (2 banks) +
        # aux 2x[128,512] (2 banks) = 8 banks exactly.
        with tc.tile_pool(name="psS", bufs=2, space="PSUM") as psS, \
             tc.tile_pool(name="psAV", bufs=2, space="PSUM") as psAV, \
             tc.tile_pool(name="psaux", bufs=2, space="PSUM") as psaux:

            # Aux matmul stream: out2 + final-projection matmuls, one PE
            # instruction per thunk, drained inside the spatial j-loops so
            # the PE always has ready work while ScalarE runs the exps.
            aux_thunks = []
            final_psf = {}

            def emit_out2(h, nb):
                p_, off = h // 2, 64 * (h % 2)
                pso = psaux.tile([128, 512], F32, tag="aux",
                                 name=f"pso{h}{nb}")
                mm = nc.tensor.matmul(
                    pso[0:64, :],
                    lhsT=secm_sb[p_][off:off + 64, :],
                    rhs=yhT[p_][off:off + 64, nb * 512:(nb + 1) * 512],
                    start=True, stop=True,
                )
                nc.vector.tensor_copy(cat4[h][:, nb * 512:(nb + 1) * 512],
                                      pso[0:64, :])
                return mm

            def emit_final_mm(d, nb, q):
                if q == 0:
                    final_psf[(d, nb)] = psaux.tile(
                        [128, 512], F32, tag="aux", name=f"psf{d}{nb}")
                psf = final_psf[(d, nb)]
                mm = nc.tensor.matmul(
                    psf[:],
                    lhsT=wq[q][:, d * 128:(d + 1) * 128],
                    rhs=cat4[q][:, nb * 512:(nb + 1) * 512],
                    start=(q == 0), stop=(q == HG - 1),
                )
                if q == HG - 1:
                    ob = opool.tile([128, 512], F32, tag="ob",
                                    name=f"ob{d}{nb}")
                    nc.vector.tensor_copy(ob[:], psf[:])
                    nc.sync.dma_start(
                        outT_d[d * 128:(d + 1) * 128,
                               nb * 512:(nb + 1) * 512],
                        ob[:],
                    )
                return mm

            for h in range(HG):
                for nb in range(4):
                    aux_thunks.append(lambda h=h, nb=nb: emit_out2(h, nb))

            def queue_finals(nb, ds=range(8)):
                for d in ds:
                    for q in range(HG):
                        aux_thunks.append(
                            lambda d=d, nb=nb, q=q: emit_final_mm(d, nb, q))

            def drain_aux(k, anchor=None):
                # anchor pins the aux matmul into this drain slot's position
                # in the PE stream - the scheduler's gap-filler otherwise
                # hoists finals into earlier windows where their cat4 inputs
                # are still several microseconds from ready (its cost model
                # underestimates RECIPROCAL ~6x)
                for _ in range(k):
                    if aux_thunks:
                        mm = aux_thunks.pop(0)()
                        if anchor is not None and mm is not None:
                            add_dep_helper(mm.ins, anchor.ins, sync=False,
                                           reason="pin aux to drain slot")

            def make_tail(p_, ib, avs, ptt_last):
                # AV for the last j-pair + normalization tails; emitted at
                # the START of the next iteration so that iteration's S
                # matmuls sit ahead of it in the PE stream (ScalarE usually
                # lags by an exp or two at iteration end).
                icol = ib * 512

                def emit():
                    for hh in range(2):
                        h = 2 * p_ + hh
                        nc.tensor.matmul(
                            avs[hh][0:DH + 1, :],
                            lhsT=xh_aug[NCH - 1][:, 65 * h:65 * h + DH + 1],
                            rhs=ptt_last[:, 512 * hh:512 * hh + 512],
                            start=False, stop=True,
                        )
                    avsbs, rcs, bcs = [], [], []
                    for hh in range(2):
                        avsb = tpool.tile([DH + 1, 512], F32, tag="avsb",
                                          name=f"avsb{p_}{ib}{hh}")
                        nc.vector.tensor_copy(avsb[:], avs[hh][0:DH + 1, :])
                        avsbs.append(avsb)
                    for hh in range(2):
                        rc = tpool.tile([1, 512], F32, tag="rc",
                                        name=f"rc{p_}{ib}{hh}")
                        nc.vector.reciprocal(rc[:], avsbs[hh][DH:DH + 1, :])
                        rcs.append(rc)
                    for hh in range(2):
                        bc = tpool.tile([64, 512], F32, tag="bc",
                                        name=f"bc{p_}{ib}{hh}")
                        nc.gpsimd.partition_broadcast(bc[:], rcs[hh][:])
                        bcs.append(bc)
                    for hh in range(2):
                        h = 2 * p_ + hh
                        tmp = tpool.tile([64, 512], F32, tag="tmp",
                                         name=f"tmp{p_}{ib}{hh}")
                        nc.vector.tensor_mul(tmp[:], avsbs[hh][0:DH, :],
                                             bcs[hh][:])
                        dst = cat4[h][:, icol:icol + 512]
                        nc.vector.tensor_add(dst, tmp[:], dst)
                return emit

            pending_tail = None
            # --- spatial attention: iterations (ib 512-block, pair),
            #     processing key chunks two at a time (j-pairs) ---
            for ib in range(4):
                for p_ in range(2):
                    # nb's cat4 block is complete once BOTH pairs' tails ran;
                    # the second pair's tails execute during (ib+1, p0), so
                    # finals(nb) join the aux queue at (ib+1, p1)
                    if p_ == 1 and ib >= 1:
                        queue_finals(ib - 1)
                    icol = ib * 512
                    avs = [psAV.tile([128, 512], F32, tag="av",
                                     name=f"av{p_}{ib}{q}") for q in range(2)]
                    ptts = [None] * NCH
                    for j in range(NCH):  # key chunks
                        spt = psS.tile([128, 1024], F32, tag="S",
                                       name=f"S{p_}{ib}{j}")
                        s_anchor = None
                        for hh in range(2):
                            off = 64 * hh
                            s_anchor = nc.tensor.matmul(
                                spt[:, 512 * hh:512 * hh + 512],
                                lhsT=yhT[p_][off:off + 64,
                                             j * 128:(j + 1) * 128],
                                rhs=z1T[p_][off:off + 64, icol:icol + 512],
                                start=True, stop=True,
                            )
                        ptt = ptpool.tile([128, 1024], ATT, tag="pt",
                                          name=f"pt{p_}{ib}{j}")
                        nc.scalar.activation(ptt[:], spt[:], EXP, scale=SCALE)
                        ptts[j] = ptt
                        if j == 0 and pending_tail is not None:
                            pending_tail()
                            pending_tail = None
                        drain_aux(1, s_anchor)
                        if j > 0:
                            for hh in range(2):
                                h = 2 * p_ + hh
                                nc.tensor.matmul(
                                    avs[hh][0:DH + 1, :],
                                    lhsT=xh_aug[j - 1][:, 65 * h:65 * h + DH + 1],
                                    rhs=ptts[j - 1][:, 512 * hh:512 * hh + 512],
                                    start=(j == 1), stop=False,
                                )
                    pending_tail = make_tail(p_, ib, avs, ptts[NCH - 1])
            pending_tail()
            queue_finals(3)
            drain_aux(len(aux_thunks))

    nc.compile()
    return nc


_NC_CACHE = {}


def _get_program():
    if "nc" not in _NC_CACHE:
        _NC_CACHE["nc"] = _build_program()
    return _NC_CACHE["nc"]


def _prep_input_maps(x, y, z, w_sa1, w_sa2, w_se1, w_se2, w_out):
    f32 = lambda a: np.ascontiguousarray(np.asarray(a, dtype=np.float32))
    bf16 = lambda a: np.ascontiguousarray(
        np.asarray(a, dtype=np.float32).astype(ml_dtypes.bfloat16))
    maps = []
    for c in range(NCORES):
        b, g = divmod(c, G)
        sl = slice(g * CIN, (g + 1) * CIN)
        maps.append({
            "xT": bf16(np.asarray(x)[b].T),
            "yT": bf16(np.asarray(y)[b].T),
            "zT": bf16(np.asarray(z)[b].T),
            "w_sa1": bf16(np.asarray(w_sa1)[:, sl]),
            "w_sa2": bf16(np.asarray(w_sa2)[:, sl]),
            "w_se1": bf16(np.asarray(w_se1)[:, sl]),
            "w_se2": bf16(np.asarray(w_se2)[:, sl]),
            "w_out": bf16(np.asarray(w_out)[sl, :]),
        })
    return maps


def run(inputs, trace=False, trace_kwargs=None):
    """Run on hardware; returns (full_output, BassKernelResults)."""
    nc = _get_program()
    in_maps = _prep_input_maps(
        inputs["x"], inputs["y"], inputs["z"],
        inputs["w_sa1"], inputs["w_sa2"], inputs["w_se1"], inputs["w_se2"],
        inputs["w_out"],
    )
    res = run_bass_kernel_spmd(
        nc, in_maps, list(range(NCORES)), trace=trace,
        trace_kwargs=trace_kwargs or {},
    )
    out = np.zeros((B, N, DIM), dtype=np.float32)
    for c in range(NCORES):
        b, _g = divmod(c, G)
        out[b] += res.results[c]["outT"].T
    out += np.asarray(inputs["b_out"], dtype=np.float32)
    return out, res


def kernel(**inputs) -> np.ndarray:
    out, _ = run(inputs, trace=False)
    return out



# revision 18
# speedup vs baseline: 1.0887x; 1.0887x over previous
"""Trainium2 Bass kernel for nn_Attention_81037442941065.

Dual-attention module (spatial [b,h,n,n] + channel [b,h,d,d]) with
B=2, N=2048, DIM=1024, 16 heads of d=64.

Sharding: 8 cores = (2 batches) x (4 head-groups of 4 heads).
Each core computes its batch/head-group slice end-to-end and produces a
partial (over head groups) output projection; the host sums the 4 group
partials per batch and adds b_out.

Key optimizations over the bf16 baseline:
  * fp8e4m3 DoubleRow matmuls (0.5 cycles/row = 2x bf16 PE throughput)
    for the x/z1/z2 projections, the channel-attention logits and the
    spatial AV matmul. Weights are pre-scaled by 32 so their ~0.02
    magnitudes stay out of the fp8 subnormal range; the 1/32 factors are
    folded into the softmax activation scales and the AV ones-row (=32).
  * DR matmuls must write psum starting at partition 0, so 64-row output
    blocks share one bank as column halves "riding" a single psum
    accumulation group (start=False writes into the zero region opened
    by the group's first start; explicit dep edges order the riders
    after the start and the stop last). Partition halves 64-127 of
    SBUF destinations are filled via small SBUF->SBUF DMAs (the only
    partition-crossing path that doesn't cost PE/ACT cycles).
  * Spatial AV runs as 4 aux units per iteration (head x q-half), each
    a 16-matmul DR chain over all 8 key-chunk-pairs, with the softmax
    denominators from a ones(=32)-row riding B-matmul at partition 0.
    Units drain in the next iteration's first slots, so no dedicated
    psum pool is needed.
  * The y projection, S logits, out2 and the final projection stay bf16:
    out2 dominates the output (|out2| ~ 5x |out1|), so the channel path
    carries the precision. cat is stored head-pair-packed so the final
    projection contracts K=128 (full PE) instead of K=64.
  * All projections run as labeled aux PE units drained inside the
    spatial S/exp/AV loop (deadline-ordered queue + drain_until guards),
    so ScalarE (exp over the full [n,n] maps, ~135us, the co-bottleneck
    with PE at ~135us) starts within ~10us and stays saturated. All
    psum->SBUF copies run on DVE, never ScalarE.
  * bf16 output partials (halves outbound DMA).

Measured end-to-end relative error ~1e-2 vs the fp32 reference (gate 2e-2).
"""

import sys

for _p in ("/opt/trn_rl_repo", "/opt/pypackages"):
    if _p not in sys.path:
        sys.path.insert(0, _p)

import ml_dtypes
import numpy as np
from contextlib import ExitStack

import concourse.bacc as bacc
import concourse.mybir as mybir
import concourse.tile as tile
from concourse.tile import add_dep_helper
from concourse.bass_utils import run_bass_kernel_spmd

F32 = mybir.dt.float32
BF16 = mybir.dt.bfloat16
FP8 = mybir.dt.float8e4
EXP = mybir.ActivationFunctionType.Exp
DR = mybir.MatmulPerfMode.DoubleRow

B, N, DIM = 2, 2048, 1024
HEADS, DH = 16, 64
G = 4              # head groups == cores per batch
HG = HEADS // G    # heads per group (4)
CIN = HG * DH      # inner channels per core (256)
NCORES = 8
NCH = N // 128     # 128-token chunks (16)
NJP = NCH // 2     # chunk pairs (8)
WS = 32.0          # fp8 weight prescale
SCALE = DH ** -0.5            # 1/8
S_EXP_SCALE = SCALE / WS      # z1 carries x32
CM_EXP_SCALE = SCALE / (N / DH) / (WS * WS)  # xh,z2 both carry x32
XP = DH + 4        # xq head pitch (64 ch + ones col @64, pad to 68)


def _ride(mm, host, why):
    add_dep_helper(mm.ins, host.ins, sync=False, reason=why)


def _build_program():
    nc = bacc.Bacc(
        "TRN2", target_bir_lowering=False, debug=False, num_devices=NCORES
    )

    # ---- DRAM I/O ----
    xT_d = nc.dram_tensor("xT", [DIM, N], FP8, kind="ExternalInput").ap()
    yT_d = nc.dram_tensor("yT", [DIM, N], BF16, kind="ExternalInput").ap()
    zT_d = nc.dram_tensor("zT", [DIM, N], FP8, kind="ExternalInput").ap()
    wsa1_d = nc.dram_tensor("w_sa1", [DIM, CIN], FP8, kind="ExternalInput").ap()
    wsa2_d = nc.dram_tensor("w_sa2", [DIM, CIN], BF16, kind="ExternalInput").ap()
    wse1_d = nc.dram_tensor("w_se1", [DIM, CIN], FP8, kind="ExternalInput").ap()
    wse2_d = nc.dram_tensor("w_se2", [DIM, CIN], FP8, kind="ExternalInput").ap()
    wout_d = nc.dram_tensor("w_out", [CIN, DIM], BF16, kind="ExternalInput").ap()
    outT_d = nc.dram_tensor("outT", [DIM, N], BF16, kind="ExternalOutput").ap()

    with tile.TileContext(nc) as tc, ExitStack() as ctx:
        ppool = ctx.enter_context(tc.tile_pool(name="persist", bufs=1))
        ipool = ctx.enter_context(tc.tile_pool(name="inputs", bufs=1))
        ptpool = ctx.enter_context(tc.tile_pool(name="pt", bufs=12))
        tpool = ctx.enter_context(tc.tile_pool(name="tails", bufs=3))
        opool = ctx.enter_context(tc.tile_pool(name="oout", bufs=4))
        psS = ctx.enter_context(tc.tile_pool(name="psS", bufs=2, space="PSUM"))
        psaux = ctx.enter_context(tc.tile_pool(name="psaux", bufs=4, space="PSUM"))

        # ---- persistent tiles ----
        z1T = [ppool.tile([128, N], BF16, tag=f"z1T{m}", name=f"z1T{m}")
               for m in range(2)]   # head pair m, 32x scaled
        yhT = [ppool.tile([128, N], BF16, tag=f"yhT{m}", name=f"yhT{m}")
               for m in range(2)]
        catp = [ppool.tile([128, N], BF16, tag=f"cat{m}", name=f"cat{m}")
                for m in range(2)]  # head-pair-packed out1+out2
        # xq[jp]: [tok128, parity, head, XP]; ch 0..63 = 32*xh, col 64 = 32.0
        xq = [ppool.tile([128, 2, HG, XP], FP8, tag=f"xq{j}", name=f"xq{j}")
              for j in range(NJP)]
        zq = [ppool.tile([128, 2, HG, DH], FP8, tag=f"zq{j}", name=f"zq{j}")
              for j in range(NJP)]
        secm_sb = [ppool.tile([128, DH], BF16, tag=f"cm{p}", name=f"cm{p}")
                   for p in range(2)]
        rs = [ppool.tile([64, 1], F32, tag=f"rs{h}", name=f"rs{h}")
              for h in range(HG)]
        rcm = [ppool.tile([64, 1], F32, tag=f"rcm{h}", name=f"rcm{h}")
               for h in range(HG)]

        # ---- input tiles ----
        # fp8 dim-pair-interleaved: [k, i, n] = src[256p + 128i + k, n]
        zt8 = [ipool.tile([128, 2, N], FP8, tag=f"zt{p}", name=f"zt{p}")
               for p in range(4)]
        xt8 = [ipool.tile([128, 2, N], FP8, tag=f"xt{p}", name=f"xt{p}")
               for p in range(4)]
        yt = [ipool.tile([128, N], BF16, tag=f"yt{k}", name=f"yt{k}")
              for k in range(8)]
        wsa1_t = [ipool.tile([128, 2, CIN], FP8, tag=f"wsa1_{p}",
                             name=f"wsa1_{p}") for p in range(4)]
        wse1_t = [ipool.tile([128, 2, CIN], FP8, tag=f"wse1_{p}",
                             name=f"wse1_{p}") for p in range(4)]
        wse2_t = [ipool.tile([128, 2, CIN], FP8, tag=f"wse2_{p}",
                             name=f"wse2_{p}") for p in range(4)]
        wsa2_t = [ipool.tile([128, CIN], BF16, tag=f"wsa2_{k}",
                             name=f"wsa2_{k}") for k in range(8)]
        wp = [ipool.tile([128, DIM], BF16, tag=f"wp{p}", name=f"wp{p}")
              for p in range(2)]

        def flat2(t):
            return t[:].rearrange("p two n -> p (two n)")

        # ---- input DMAs, in consumption order ----
        for p in range(4):
            for i in range(2):
                r = 256 * p + 128 * i
                nc.sync.dma_start(flat2(wsa1_t[p])[:, CIN * i:CIN * (i + 1)],
                                  wsa1_d[r:r + 128, :])
        for k in range(8):
            nc.sync.dma_start(wsa2_t[k][:], wsa2_d[k * 128:(k + 1) * 128, :])
        for p in range(4):
            for i in range(2):
                r = 256 * p + 128 * i
                nc.sync.dma_start(flat2(zt8[p])[:, N * i:N * (i + 1)],
                                  zT_d[r:r + 128, :])
        for k in range(8):
            nc.sync.dma_start(yt[k][:], yT_d[k * 128:(k + 1) * 128, :])
        for p in range(4):
            for i in range(2):
                r = 256 * p + 128 * i
                nc.sync.dma_start(flat2(wse1_t[p])[:, CIN * i:CIN * (i + 1)],
                                  wse1_d[r:r + 128, :])
                nc.sync.dma_start(flat2(wse2_t[p])[:, CIN * i:CIN * (i + 1)],
                                  wse2_d[r:r + 128, :])
        for p in range(4):
            for i in range(2):
                r = 256 * p + 128 * i
                nc.sync.dma_start(flat2(xt8[p])[:, N * i:N * (i + 1)],
                                  xT_d[r:r + 128, :])
        for p in range(2):
            nc.sync.dma_start(wp[p][:], wout_d[p * 128:(p + 1) * 128, :])

        # catp starts at 0 (out1/out2 both accumulate); xq ones columns
        for m in range(2):
            nc.gpsimd.memset(catp[m][:], 0.0)
        for j in range(NJP):
            nc.gpsimd.memset(xq[j][:, :, :, DH:DH + 1], WS)

        # ================= aux PE unit emitters =================
        def lhs_xq(jp, h, c0, c1):
            # [128, 2, c1-c0] fp8 slice of xq for head h
            return xq[jp][:].rearrange(
                "p two h c -> p two (h c)")[:, :, XP * h + c0:XP * h + c1]

        def lhs_zq(jp, h):
            return zq[jp][:].rearrange(
                "p two h c -> p two (h c)")[:, :, DH * h:DH * (h + 1)]

        def emit_z1T(c, nb):
            # z1T 64-ch block c, 512-token block nb (fp8 DR).
            # Two token halves t2 ride one psum group as column halves.
            ps = psaux.tile([128, 512], F32, tag="aux", name=f"z1p{c}{nb}")
            host = {}
            for t2 in range(2):
                for p in range(4):
                    mm = nc.tensor.matmul(
                        ps[0:64, 256 * t2:256 * t2 + 256],
                        lhsT=wsa1_t[p][:, :, 64 * c:64 * c + 64],
                        rhs=zt8[p][:, :, 512 * nb + 256 * t2:
                                   512 * nb + 256 * t2 + 256],
                        start=(t2 == 0 and p == 0),
                        stop=(t2 == 1 and p == 3), perf_mode=DR,
                        skip_group_check=True,
                    )
                    host[(t2, p)] = mm
                    if t2 == 1 and p == 0:
                        _ride(mm, host[(0, 0)], "rider after group start")
                    if t2 == 1 and p == 3:
                        _ride(mm, host[(0, 3)], "stop after host chain")
            m, off = c // 2, 64 * (c % 2)
            if off == 0:
                nc.vector.tensor_copy(
                    z1T[m][0:64, 512 * nb:512 * nb + 512], ps[0:64, :])
            else:
                stg = tpool.tile([64, 512], BF16, tag="z1stg",
                                 name=f"z1stg{c}{nb}")
                nc.vector.tensor_copy(stg[:], ps[0:64, :])
                nc.sync.dma_start(
                    z1T[m][64:128, 512 * nb:512 * nb + 512], stg[:])
            return mm

        def emit_yhT(m, nb, kq):
            # yhT pair m, token block nb, contraction quarter kq (bf16)
            if kq == 0:
                emit_yhT.ps[(m, nb)] = psaux.tile(
                    [128, 512], F32, tag="aux", name=f"yhp{m}{nb}")
            ps = emit_yhT.ps[(m, nb)]
            mm = None
            for k in (2 * kq, 2 * kq + 1):
                mm = nc.tensor.matmul(
                    ps[:],
                    lhsT=wsa2_t[k][:, 128 * m:128 * m + 128],
                    rhs=yt[k][:, 512 * nb:512 * nb + 512],
                    start=(k == 0), stop=(k == 7),
                )
            if kq == 3:
                nc.vector.tensor_copy(
                    yhT[m][:, 512 * nb:512 * nb + 512], ps[:])
                del emit_yhT.ps[(m, nb)]
            return mm
        emit_yhT.ps = {}

        def _proj_dr_chunk(ps, i, lhs_tiles, rhs_tiles):
            # token chunk i -> [64 tok, 256 ch] col halves riding one group
            host = {}
            mm = None
            for t2 in range(2):
                for p in range(4):
                    mm = nc.tensor.matmul(
                        ps[0:64, 256 * t2:256 * t2 + 256],
                        lhsT=lhs_tiles[p][:, :, 128 * i + 64 * t2:
                                          128 * i + 64 * t2 + 64],
                        rhs=rhs_tiles[p][:, :, :],
                        start=(t2 == 0 and p == 0),
                        stop=(t2 == 1 and p == 3), perf_mode=DR,
                        skip_group_check=True,
                    )
                    host[(t2, p)] = mm
                    if t2 == 1 and p == 0:
                        _ride(mm, host[(0, 0)], "rider after group start")
                    if t2 == 1 and p == 3:
                        _ride(mm, host[(0, 3)], "stop after host chain")
            return mm

        def emit_xh(i):
            # xh chunk i -> xq[i//2][:, i%2] (fp8 DR, 32x scaled); tokens
            # 64-127 bridged via SBUF->SBUF DMA (DR dst must be base 0).
            ps = psaux.tile([128, 512], F32, tag="aux", name=f"xhp{i}")
            mm = _proj_dr_chunk(ps, i, xt8, wse1_t)
            par = i % 2
            lo = ps[0:64, 0:256].rearrange("p (o h c) -> p o h c", o=1, c=DH)
            nc.vector.tensor_copy(xq[i // 2][0:64, par:par + 1, :, 0:DH], lo)
            stg = tpool.tile([64, HG, DH], FP8, tag="xstg", name=f"xstg{i}")
            hi = ps[0:64, 256:512].rearrange("p (h c) -> p h c", c=DH)
            nc.vector.tensor_copy(stg[:], hi)
            nc.sync.dma_start(xq[i // 2][64:128, par:par + 1, :, 0:DH],
                              stg[:].rearrange("p h c -> p (h c)").rearrange("p (o f) -> p o f", o=1))
            return mm

        def emit_z2(i):
            ps = psaux.tile([128, 512], F32, tag="aux", name=f"z2p{i}")
            mm = _proj_dr_chunk(ps, i, zt8, wse2_t)
            par = i % 2
            lo = ps[0:64, 0:256].rearrange("p (o h c) -> p o h c", o=1, c=DH)
            nc.vector.tensor_copy(zq[i // 2][0:64, par:par + 1, :, :], lo)
            stg = tpool.tile([64, HG, DH], FP8, tag="zstg", name=f"zstg{i}")
            hi = ps[0:64, 256:512].rearrange("p (h c) -> p h c", c=DH)
            nc.vector.tensor_copy(stg[:], hi)
            nc.sync.dma_start(zq[i // 2][64:128, par:par + 1, :, :],
                              stg[:].rearrange("p h c -> p (h c)").rearrange("p (o f) -> p o f", o=1))
            return mm

        def emit_channel():
            # channel-attn logits: all 4 heads ride ONE psum group
            # (rows 0-63, col block 64h per head) + softmax.
            cmp_ = psaux.tile([128, 512], F32, tag="aux", name="cmps")
            start_mm = None
            chain_last = {}
            mm = None
            for jp in range(NJP):
                for h in range(HG):
                    mm = nc.tensor.matmul(
                        cmp_[0:64, 64 * h:64 * h + 64],
                        lhsT=lhs_xq(jp, h, 0, DH),
                        rhs=lhs_zq(jp, h),
                        start=(jp == 0 and h == 0),
                        stop=(jp == NJP - 1 and h == HG - 1), perf_mode=DR,
                        skip_group_check=True,
                    )
                    if jp == 0 and h == 0:
                        start_mm = mm
                    elif jp == 0:
                        _ride(mm, start_mm, "rider after group start")
                    if jp == NJP - 1 and h < HG - 1:
                        chain_last[h] = mm
            for h in range(HG - 1):
                _ride(mm, chain_last[h], "stop after rider chains")
            for h in range(HG):
                p_, off = h // 2, 64 * (h % 2)
                st = tpool.tile([64, DH], BF16, tag="cmstage",
                                name=f"cmstage{h}")
                nc.scalar.activation(st[:], cmp_[0:64, 64 * h:64 * h + 64],
                                     EXP, scale=CM_EXP_SCALE,
                                     accum_out=rs[h][0:64, 0:1])
                nc.vector.reciprocal(rcm[h][0:64, 0:1], rs[h][0:64, 0:1])
                nc.vector.tensor_scalar_mul(st[:], st[:], rcm[h][0:64, 0:1])
                nc.sync.dma_start(secm_sb[p_][off:off + 64, :], st[:])
            return mm

        def emit_out2(h, nb):
            p_, off = h // 2, 64 * (h % 2)
            pso = psaux.tile([128, 512], F32, tag="aux", name=f"pso{h}{nb}")
            mm = nc.tensor.matmul(
                pso[off:off + 64, :],
                lhsT=secm_sb[p_][off:off + 64, :],
                rhs=yhT[p_][off:off + 64, nb * 512:(nb + 1) * 512],
                start=True, stop=True,
            )
            dst = catp[p_][off:off + 64, nb * 512:(nb + 1) * 512]
            nc.vector.tensor_add(dst, pso[off:off + 64, :], dst)
            return mm

        final_psf = {}

        def emit_final(d, nb, q):
            if q == 0:
                final_psf[(d, nb)] = psaux.tile(
                    [128, 512], F32, tag="aux", name=f"psf{d}{nb}")
            psf = final_psf[(d, nb)]
            mm = nc.tensor.matmul(
                psf[:],
                lhsT=wp[q][:, d * 128:(d + 1) * 128],
                rhs=catp[q][:, nb * 512:(nb + 1) * 512],
                start=(q == 0), stop=(q == 1),
            )
            if q == 1:
                ob = opool.tile([128, 512], BF16, tag="ob", name=f"ob{d}{nb}")
                nc.vector.tensor_copy(ob[:], psf[:])
                nc.sync.dma_start(
                    outT_d[d * 128:(d + 1) * 128, nb * 512:(nb + 1) * 512],
                    ob[:],
                )
            return mm

        # ---- AV units: one per (head-half hh, q-half qc) per iteration.
        # Bank layout: A-group rows 0-63 (the 64 out1 channels) cols 0:256
        # + ones(=32)-row denominator B riding at partition 0 cols 256:512.
        av_banks = {}

        def emit_av_unit(p_, ib, hh, qc, pts):
            h = 2 * p_ + hh
            bank = psaux.tile([128, 512], F32, tag="aux",
                              name=f"av{p_}{ib}{hh}{qc}")
            av_banks[(hh, qc)] = bank
            a_start = None
            a_last = None
            mm = None
            for jp in range(NJP):
                rhs = pts[jp][:, :, 512 * hh + 256 * qc:
                              512 * hh + 256 * qc + 256]
                a = nc.tensor.matmul(
                    bank[0:64, 0:256],
                    lhsT=lhs_xq(jp, h, 0, DH), rhs=rhs,
                    start=(jp == 0), stop=False, perf_mode=DR,
                    skip_group_check=True,
                )
                b = nc.tensor.matmul(
                    bank[0:1, 256:512],
                    lhsT=lhs_xq(jp, h, DH, DH + 1), rhs=rhs,
                    start=False, stop=(jp == NJP - 1), perf_mode=DR,
                    skip_group_check=True,
                )
                if jp == 0:
                    a_start = a
                    if cur_anchor[0] is not None:
                        _ride(a_start, cur_anchor[0], "pin av unit to slot")
                    _ride(b, a, "denom B after group start")
                if jp == NJP - 1:
                    a_last = a
                    _ride(b, a_last, "stop after host chain")
                mm = b
            if qc == 1:
                _emit_av_tail(p_, ib, hh)
            return mm

        def _emit_av_tail(p_, ib, hh):
            icol = ib * 512
            b0, b1 = av_banks.pop((hh, 0)), av_banks.pop((hh, 1))
            rc = tpool.tile([1, 512], F32, tag="rc", name=f"rc{p_}{ib}{hh}")
            for qc, bk in ((0, b0), (1, b1)):
                nc.vector.reciprocal(rc[0:1, 256 * qc:256 * qc + 256],
                                     bk[0:1, 256:512])
            avsb = tpool.tile([64, 512], F32, tag="avsb",
                              name=f"avsb{p_}{ib}{hh}")
            for qc, bk in ((0, b0), (1, b1)):
                nc.vector.tensor_copy(avsb[:, 256 * qc:256 * qc + 256],
                                      bk[0:64, 0:256])
            bc = tpool.tile([64, 512], F32, tag="bc", name=f"bc{p_}{ib}{hh}")
            nc.gpsimd.partition_broadcast(bc[:], rc[:])
            if hh == 0:
                tmp = tpool.tile([64, 512], F32, tag="tmp",
                                 name=f"tmp{p_}{ib}{hh}")
                nc.vector.tensor_mul(tmp[:], avsb[:], bc[:])
                dst = catp[p_][0:64, icol:icol + 512]
                nc.vector.tensor_add(dst, tmp[:], dst)
            else:
                # partitions 64-127 of catp: bridge via SBUF->SBUF DMA
                tmpb = tpool.tile([64, 512], BF16, tag="tmpb",
                                  name=f"tmpb{p_}{ib}")
                nc.vector.tensor_mul(tmpb[:], avsb[:], bc[:])
                hstage = tpool.tile([128, 512], BF16, tag="hstg",
                                    name=f"hstg{p_}{ib}")
                nc.sync.dma_start(hstage[64:128, :], tmpb[:])
                dst = catp[p_][64:128, icol:icol + 512]
                nc.vector.tensor_add(dst, hstage[64:128, :], dst)

        # ---- labeled aux queue (drained inside the spatial j-slots) ----
        # Emission order IS a correctness constraint: Tile only sees writes
        # that were already emitted, so consumers force their producers out
        # of the queue with drain_until() before touching the data.
        aux_thunks = []
        aux_done = set()
        cur_anchor = [None]

        def queue(label, fn, *args):
            aux_thunks.append((label, lambda fn=fn, args=args: fn(*args)))

        def pop_one():
            label, thunk = aux_thunks.pop(0)
            mm = thunk()
            aux_done.add(label)
            if cur_anchor[0] is not None and mm is not None:
                add_dep_helper(mm.ins, cur_anchor[0].ins, sync=False,
                               reason="pin aux to drain slot")

        def drain_aux(k):
            for _ in range(k):
                if aux_thunks:
                    pop_one()

        def drain_until(label):
            while label not in aux_done and aux_thunks:
                pop_one()

        # prologue: just enough for S(p_=0, ib=0, j<4) to start
        emit_z1T(0, 0)
        emit_z1T(1, 0)
        aux_done.add(("z1c", 1, 0))
        for kq in range(4):
            emit_yhT(0, 0, kq)
        aux_done.add(("yhT", 0, 0))

        # deadline order for p_-outer iteration: xh pairs feed AV units of
        # (p0,ib0) drained at (p0,ib1) slots 0-3; yhT m0 blocks feed S(p0,*)
        # keys; all m1 projections are only needed from slot 64 (p_=1).
        queue(("xh", 0), emit_xh, 0)
        queue(("xh", 1), emit_xh, 1)
        ym0 = [(0, nbb, kqq) for nbb in (1, 2, 3) for kqq in range(4)]
        yi = 0
        for i in range(2, 16):
            queue(("xh", i), emit_xh, i)
            if yi < len(ym0):
                for u in ym0[yi:yi + 2]:
                    queue(("yhT", u[0], u[1]) if u[2] == 3 else
                          ("yhTk",) + u, emit_yhT, *u)
                yi += 2
        while yi < len(ym0):
            u = ym0[yi]
            queue(("yhT", u[0], u[1]) if u[2] == 3 else ("yhTk",) + u,
                  emit_yhT, *u)
            yi += 1
        queue(("z1c", 0, 1), emit_z1T, 0, 1)
        queue(("z1c", 1, 1), emit_z1T, 1, 1)
        for i in range(8):
            queue(("z2", i), emit_z2, i)
        queue(("z1c", 0, 2), emit_z1T, 0, 2)
        queue(("z1c", 1, 2), emit_z1T, 1, 2)
        for i in range(8, 16):
            queue(("z2", i), emit_z2, i)
        for nbb in range(4):
            for kqq in range(4):
                queue(("yhT", 1, nbb) if kqq == 3 else ("yhTk", 1, nbb, kqq),
                      emit_yhT, 1, nbb, kqq)
        queue(("z1c", 0, 3), emit_z1T, 0, 3)
        queue(("z1c", 1, 3), emit_z1T, 1, 3)
        queue(("z1c", 2, 0), emit_z1T, 2, 0)
        queue(("z1c", 3, 0), emit_z1T, 3, 0)
        queue(("ch",), emit_channel)
        queue(("z1c", 2, 1), emit_z1T, 2, 1)
        queue(("z1c", 3, 1), emit_z1T, 3, 1)
        for h in range(HG):
            queue(("out2", h, 0), emit_out2, h, 0)
        queue(("z1c", 2, 2), emit_z1T, 2, 2)
        queue(("z1c", 3, 2), emit_z1T, 3, 2)
        for nb in range(1, 4):
            for h in range(HG):
                queue(("out2", h, nb), emit_out2, h, nb)
        queue(("z1c", 2, 3), emit_z1T, 2, 3)
        queue(("z1c", 3, 3), emit_z1T, 3, 3)

        def queue_finals(nb):
            drain_until(("out2", HG - 1, nb))
            for d in range(8):
                for q in range(2):
                    queue(("fin", d, nb, q), emit_final, d, nb, q)

        # ================= spatial attention =================
        pt = {}             # jp -> exp tile for current iteration
        pending_av = []     # AV unit thunks from the previous iteration

        first_iter = True
        for p_ in range(2):
            for ib in range(4):
                icol = ib * 512
                drain_until(("z1c", 2 * p_ + 1, ib))
                for j in range(NCH):
                    jp, par = j // 2, j % 2
                    if par == 0:
                        drain_until(("yhT", p_, j // 4))
                    spt = psS.tile([128, 1024], F32, tag="S",
                                   name=f"S{p_}{ib}{j}")
                    s_anchor = None
                    for hh in range(2):
                        off = 64 * hh
                        s_anchor = nc.tensor.matmul(
                            spt[:, 512 * hh:512 * hh + 512],
                            lhsT=yhT[p_][off:off + 64,
                                         j * 128:(j + 1) * 128],
                            rhs=z1T[p_][off:off + 64, icol:icol + 512],
                            start=True, stop=True,
                        )
                    cur_anchor[0] = s_anchor
                    if par == 0:
                        pt[jp] = ptpool.tile(
                            [128, 2, 1024], FP8, tag="pt",
                            name=f"pt{p_}{ib}{jp}")
                    nc.scalar.activation(
                        flat2(pt[jp])[:, 1024 * par:1024 * (par + 1)],
                        spt[:], EXP, scale=S_EXP_SCALE)
                    if pending_av:
                        pending_av.pop(0)()
                    elif j == 4 and p_ == 1 and ib >= 1:
                        queue_finals(ib - 1)
                        drain_aux(1)
                    else:
                        drain_aux(2 if (first_iter or
                                        len(aux_thunks) > 56) else 1)
                pts = [pt[jp] for jp in range(NJP)]
                pending_av = [
                    lambda p_=p_, ib=ib, hh=hh, qc=qc, pts=pts:
                        emit_av_unit(p_, ib, hh, qc, pts)
                    for hh in range(2) for qc in range(2)]
                drain_until(("xh", 15))
                first_iter = False
        cur_anchor[0] = None
        for th in pending_av:
            th()
        queue_finals(3)
        drain_aux(len(aux_thunks))

    nc.compile()
    return nc


_NC_CACHE = {}


def _get_program():
    if "nc" not in _NC_CACHE:
        _NC_CACHE["nc"] = _build_program()
    return _NC_CACHE["nc"]


def _prep_input_maps(x, y, z, w_sa1, w_sa2, w_se1, w_se2, w_out):
    bf16 = lambda a: np.ascontiguousarray(
        np.asarray(a, dtype=np.float32).astype(ml_dtypes.bfloat16))
    fp8 = lambda a: np.ascontiguousarray(
        np.asarray(a, dtype=np.float32).astype(ml_dtypes.float8_e4m3))
    maps = []
    for c in range(NCORES):
        b, g = divmod(c, G)
        sl = slice(g * CIN, (g + 1) * CIN)
        maps.append({
            "xT": fp8(np.asarray(x)[b].T),
            "yT": bf16(np.asarray(y)[b].T),
            "zT": fp8(np.asarray(z)[b].T),
            "w_sa1": fp8(np.asarray(w_sa1)[:, sl] * WS),
            "w_sa2": bf16(np.asarray(w_sa2)[:, sl]),
            "w_se1": fp8(np.asarray(w_se1)[:, sl] * WS),
            "w_se2": fp8(np.asarray(w_se2)[:, sl] * WS),
            "w_out": bf16(np.asarray(w_out)[sl, :]),
        })
    return maps


def run(inputs, trace=False, trace_kwargs=None):
    """Run on hardware; returns (full_output, BassKernelResults)."""
    nc = _get_program()
    in_maps = _prep_input_maps(
        inputs["x"], inputs["y"], inputs["z"],
        inputs["w_sa1"], inputs["w_sa2"], inputs["w_se1"], inputs["w_se2"],
        inputs["w_out"],
    )
    res = run_bass_kernel_spmd(
        nc, in_maps, list(range(NCORES)), trace=trace,
        trace_kwargs=trace_kwargs or {},
    )
    out = np.zeros((B, N, DIM), dtype=np.float32)
    for c in range(NCORES):
        b, _g = divmod(c, G)
        out[b] += np.asarray(res.results[c]["outT"], dtype=np.float32).T
    out += np.asarray(inputs["b_out"], dtype=np.float32)
    return out, res


def kernel(**inputs) -> np.ndarray:
    out, _ = run(inputs, trace=False)
    return out
